# revision 1
# baseline (speedup 1.0000x reference)
"""Trainium2 Bass kernel for Transformer-XL style relative-position multi-head
self-attention (nn_MultiHeadedSelfAttention_35588099015524).

Sharding: batch (B=8) is data-parallel across the 8 NeuronCores; no collectives.

Math trick: the Transformer-XL relative shift is eliminated exactly via
    sin(w(j-i)) = sin(wj)cos(wi) - cos(wj)sin(wi)
    cos(w(j-i)) = cos(wj)cos(wi) + sin(wj)sin(wi)
so  matrix_bd[i,j] = sum_c A'[i,c] * F[j,c]   (plain matmul, K=512)
where G = q_v @ Wpos_h^T (per head), A' is a RoPE-style per-row rotation of G
(elementwise with constant sin/cos tables), and F is the constant sinusoid
table at positions 0..T-1.  Scores are computed transposed (S^T[j,i]) so that
softmax-normalization sums ride along as an extra ones-column in V, and the
attention output and final projection need no on-device transposes at all.
"""

import sys

sys.path.insert(0, "/opt/trn_rl_repo")

from contextlib import ExitStack  # noqa: E402

import numpy as np  # noqa: E402
import ml_dtypes  # noqa: E402

import concourse.bass as bass  # noqa: E402
from concourse import bacc, library_config  # noqa: E402
import concourse.tile as tile  # noqa: E402
from concourse import mybir  # noqa: E402
from concourse.bass_utils import run_bass_kernel_spmd  # noqa: E402

# Force every ACT function we use (Exp/Ln/Copy) to resolve to the single
# "natural_log_exp_and_others" table set — otherwise the table-load pass
# flip-flops between sets per head (~2.7us per ACT_TABLE_LOAD).
import concourse.hw_specs as _hs  # noqa: E402
import concourse.bacc as _bacc_mod  # noqa: E402

if not getattr(_hs, "_act_tables_pinned", False):
    _orig_gat = _hs.get_activation_tables

    def _pinned_gat(arch):
        tabs = _orig_gat(arch)
        keep = "natural_log_exp_and_others"
        pin = {mybir.ActivationFunctionType.Exp,
               mybir.ActivationFunctionType.Ln,
               mybir.ActivationFunctionType.Copy}
        if keep in tabs and pin <= tabs[keep]:
            for k in tabs:
                if k != keep:
                    tabs[k] = tabs[k] - pin
        return tabs

    _hs.get_activation_tables = _pinned_gat
    _bacc_mod.get_activation_tables = _pinned_gat
    _hs._act_tables_pinned = True

B, T, D = 8, 1024, 512
H, DH = 8, 64
NCORES = 8
SCALE = 1.0 / np.sqrt(DH)

F32 = mybir.dt.float32
BF16 = mybir.dt.bfloat16

# knob: matmul/elementwise working dtype ("bf16" or "f32r")
MM_MODE = "bf16"


def _np_dt(mode):
    return ml_dtypes.bfloat16 if mode == "bf16" else np.float32


def _mm_dt(mode):
    return BF16 if mode == "bf16" else mybir.dt.float32r


def build_nc(mode=MM_MODE):
    """Build the per-core Bass module (identical program on all 8 cores)."""
    DT = _mm_dt(mode)
    nc = bacc.Bacc("TRN2", target_bir_lowering=False, debug=False)

    # ---- DRAM parameters (per core) ----
    xsT_d = nc.declare_dram_parameter("xsT", [D, T], DT, isOutput=False)
    wq_d = nc.declare_dram_parameter("Wq", [D, D], DT, isOutput=False)
    wk_d = nc.declare_dram_parameter("Wk", [D, D], DT, isOutput=False)
    wv_d = nc.declare_dram_parameter("Wv", [D, D], DT, isOutput=False)
    wpt_d = nc.declare_dram_parameter("WPT", [2 * D, D], DT, isOutput=False)
    wout_d = nc.declare_dram_parameter("Wout", [D, D], DT, isOutput=False)
    ubt_d = nc.declare_dram_parameter("ubT", [128, 4], F32, isOutput=False)
    vbt_d = nc.declare_dram_parameter("vbT", [128, 4], F32, isOutput=False)
    ft_d = nc.declare_dram_parameter("FT", [D, T], DT, isOutput=False)
    ct_d = nc.declare_dram_parameter("CT", [256, T], DT, isOutput=False)
    st_d = nc.declare_dram_parameter("ST", [256, T], DT, isOutput=False)
    out_d = nc.declare_dram_parameter("out", [T, D], F32, isOutput=True)

    Exp = mybir.ActivationFunctionType.Exp
    Copy = mybir.ActivationFunctionType.Copy
    MUL = mybir.AluOpType.mult
    ADD = mybir.AluOpType.add
    SUB = mybir.AluOpType.subtract

    with tile.TileContext(nc) as tc, ExitStack() as ctx:
        cpool = ctx.enter_context(tc.tile_pool(name="consts", bufs=1))
        gpool = ctx.enter_context(tc.tile_pool(name="gwork", bufs=2))
        apool = ctx.enter_context(tc.tile_pool(name="attn", bufs=2))
        opool = ctx.enter_context(tc.tile_pool(name="osb", bufs=4))
        rpool = ctx.enter_context(tc.tile_pool(name="recip", bufs=2))
        ps_s = ctx.enter_context(tc.tile_pool(name="ps_s", bufs=3, space="PSUM"))
        ps_g = ctx.enter_context(tc.tile_pool(name="ps_g", bufs=3, space="PSUM"))
        ps_z = ctx.enter_context(tc.tile_pool(name="ps_z", bufs=2, space="PSUM"))

        # ---- load constants / inputs into SBUF ----
        # one wide tile per tensor, one coalesced DMA (blocks along free dim)
        def load_wide(dram, rows, cols, tag):
            nblk = rows // 128
            t = cpool.tile([128, nblk * cols], DT, tag=tag, name=tag)
            nc.sync.dma_start(
                t[:].rearrange("p (c i) -> p c i", c=nblk),
                dram[:, :].rearrange("(c p) i -> p c i", p=128))
            return [t[:, c * cols:(c + 1) * cols] for c in range(nblk)]

        # PE warm-up during the input-DMA window: 12 dependency-free matmuls
        # all writing ONE psum tile (WAW keeps them in-order on PE; no pool
        # churn), so HAM reaches 8/8 before the first real matmul
        warm = cpool.tile([128, 512], DT, tag="warm", name="warm")
        nc.vector.memset(warm[:], 0.0)
        wp = ps_z.tile([128, 512], F32, tag="z", name="warmp")
        for w in range(12):
            nc.tensor.matmul(wp[:], warm[:, 0:128], warm[:], start=True,
                             stop=True)

        # interleave the first chunks of xsT and Wq so the first projection
        # matmul can issue as early as possible
        xsT_tile = cpool.tile([128, 4 * T], DT, tag="xsT", name="xsT")
        wq_tile = cpool.tile([128, 4 * D], DT, tag="wq", name="wq")
        for c in range(4):
            nc.sync.dma_start(xsT_tile[:, c * T:(c + 1) * T],
                              xsT_d[c * 128:(c + 1) * 128, :])
            nc.sync.dma_start(wq_tile[:, c * D:(c + 1) * D],
                              wq_d[c * 128:(c + 1) * 128, :])
        xsT = [xsT_tile[:, c * T:(c + 1) * T] for c in range(4)]
        wq = [wq_tile[:, c * D:(c + 1) * D] for c in range(4)]
        ubt = cpool.tile([128, 4], F32, tag="ubt")
        nc.sync.dma_start(ubt[:], ubt_d[:, :])
        vbt = cpool.tile([128, 4], F32, tag="vbt")
        nc.sync.dma_start(vbt[:], vbt_d[:, :])
        wpt = load_wide(wpt_d, 2 * D, D, "wpt")
        wk = load_wide(wk_d, D, D, "wk")
        wv = load_wide(wv_d, D, D, "wv")
        ct = load_wide(ct_d, 256, T, "ct")
        st = load_wide(st_d, 256, T, "st")
        ft = load_wide(ft_d, D, T, "ft")
        wout = load_wide(wout_d, D, D, "wout")

        # computed persistent tensors
        quT = [cpool.tile([128, T], DT, tag=f"quT{c}", name=f"quT{c}") for c in range(4)]
        qvT = [cpool.tile([128, T], DT, tag=f"qvT{c}", name=f"qvT{c}") for c in range(4)]
        ktp = cpool.tile([128, H * T], DT, tag="ktp", name="ktp")
        zT = [cpool.tile([128, T], DT, tag=f"zT{c}", name=f"zT{c}") for c in range(4)]
        vp = cpool.tile([128, 8 * 520], DT, tag="vp")

        # gpsimd ucode library providing InstPartitionBroadcast
        nc.gpsimd.load_library(library_config.attn)
        # ones columns for the softmax-sum trick (V gets overwritten on top)
        nc.gpsimd.memset(vp[:], 1.0)

        # ---- projections ----
        # Q^T[n,i] = sum_d Wq[d,n] xsT[d,i]   (psum tile per (n-chunk, i-chunk))
        for nchunk in range(4):
            for icnk in range(2):
                p = ps_s.tile([128, 512], F32, tag="s")
                for kc in range(4):
                    nc.tensor.matmul(
                        p[:],
                        wq[kc][:, nchunk * 128:(nchunk + 1) * 128],
                        xsT[kc][:, icnk * 512:(icnk + 1) * 512],
                        start=(kc == 0),
                        stop=(kc == 3),
                    )
                dst = quT[nchunk][:, icnk * 512:(icnk + 1) * 512]
                nc.vector.tensor_scalar_add(dst, p[:], ubt[:, nchunk:nchunk + 1])
                dst = qvT[nchunk][:, icnk * 512:(icnk + 1) * 512]
                nc.vector.tensor_scalar_add(dst, p[:], vbt[:, nchunk:nchunk + 1])

        # K^T per head, zero-padded to 128 contraction rows (so score matmuls
        # never change the stationary row count); head h's 64 real rows sit at
        # partitions (h%2)*64 to match quT/qvT tile row coordinates.
        nc.gpsimd.memset(ktp[:], 0.0)
        for h in range(H):
            row = (h % 2) * 64
            for jc in range(2):
                p = ps_s.tile([128, 512], F32, tag="s")
                for kc in range(4):
                    nc.tensor.matmul(
                        p[row:row + 64, :],
                        wk[kc][:, h * 64:(h + 1) * 64],
                        xsT[kc][:, jc * 512:(jc + 1) * 512],
                        start=(kc == 0),
                        stop=(kc == 3),
                    )
                dst = ktp[row:row + 64, h * 1024 + jc * 512:
                          h * 1024 + jc * 512 + 512]
                nc.scalar.activation(dst, p[row:row + 64, :], Copy)

        # V[j,n] = sum_d xsT[d,j] Wv[d,n]; store with stride 65 into vp
        for jt in range(8):
            p = ps_s.tile([128, 512], F32, tag="s")
            for kc in range(4):
                nc.tensor.matmul(
                    p[:],
                    xsT[kc][:, jt * 128:(jt + 1) * 128],
                    wv[kc][:],
                    start=(kc == 0),
                    stop=(kc == 3),
                )
            dst = vp[:, jt * 520:(jt + 1) * 520].rearrange(
                "p (h x) -> p h x", h=8)[:, :, 0:64]
            src = p[:].rearrange("p (h x) -> p h x", h=8)
            nc.scalar.activation(dst, src, Copy)

        # ---- per-head G -> rope(A') pipeline ----
        def emit_g_piece(h, g, piece):
            """One G^T matmul (of 8) for head h; piece = cc*2 + icnk."""
            row = (h % 2) * 64
            cc, icnk = piece // 2, piece % 2
            p = ps_g.tile([128, 512], F32, tag="g", name="gp")
            nc.tensor.matmul(
                p[:],
                wpt[h][:, cc * 128:(cc + 1) * 128],
                qvT[h // 2][:, icnk * 512:(icnk + 1) * 512],
                start=True,
                stop=True,
            )
            nc.scalar.activation(
                g[:, cc * 1024 + icnk * 512: cc * 1024 + icnk * 512 + 512],
                p[:], Copy)

        def emit_g(h):
            g = gpool.tile([128, 4096], DT, tag="g")
            for piece in range(8):
                emit_g_piece(h, g, piece)
            return g

        def emit_rope(h, g):
            ap = gpool.tile([128, 4096], DT, tag="aprime")
            tmp = gpool.tile([128, T], DT, tag="tmp1")
            tmp2 = gpool.tile([128, T], DT, tag="tmp2")
            for cc in range(4):
                freq = cc % 2          # which 128-block of frequencies
                sin_blk = cc < 2       # first half pairs with sin(w j)
                ga = g[:, cc * 1024:(cc + 1) * 1024]
                # partner chunk: cc+2 mod 4 (sin block <-> cos block)
                pc = (cc + 2) % 4
                gb = g[:, pc * 1024:(pc + 1) * 1024]
                dst = ap[:, cc * 1024:(cc + 1) * 1024]
                # sin block: A' = Gs*cos + Gc*sin ; cos block: A' = Gc*cos - Gs*sin
                nc.vector.tensor_tensor(tmp[:], ga, ct[freq][:], op=MUL)
                nc.vector.tensor_tensor(tmp2[:], gb, st[freq][:], op=MUL)
                nc.vector.tensor_tensor(dst, tmp[:], tmp2[:],
                                        op=(ADD if sin_blk else SUB))
            return ap

        g_cur = emit_g(0)
        a_cur = emit_rope(0, g_cur)
        g_cur_next = emit_g(1)

        def emit_av_mm(h, icnk, jt, zp, attnT):
            nc.tensor.matmul(
                zp[:],
                vp[:, jt * 520 + 65 * h: jt * 520 + 65 * h + 65],
                attnT[:, jt * 1024 + icnk * 512:
                      jt * 1024 + icnk * 512 + 512],
                start=(jt == 0),
                stop=(jt == 7),
            )

        Ln = mybir.ActivationFunctionType.Ln

        def emit_znorm(h, icnk, zp):
            # 1/s computed as exp(-ln s) on ACT (DVE reciprocal is 8 cyc/elem)
            row = (h % 2) * 64
            lns = rpool.tile([1, 512], F32, tag="lns")
            nc.scalar.activation(lns[:], zp[64:65, :], Ln)
            rec = rpool.tile([1, 512], F32, tag="rec")
            nc.scalar.activation(rec[:], lns[:], Exp, scale=-1.0)
            recb = rpool.tile([64, 512], F32, tag="recb")
            nc.gpsimd.partition_broadcast(recb[:], rec[0:1, :])
            dst = zT[h // 2][row:row + 64, icnk * 512:(icnk + 1) * 512]
            nc.vector.tensor_tensor(dst, zp[0:64, :], recb[:], op=MUL)

        for h in range(H):
            row = (h % 2) * 64
            attnT = apool.tile([128, 8192], DT, tag="attnT")
            zp0 = ps_z.tile([65, 512], F32, tag="z", name="zp0")
            zp1 = ps_z.tile([65, 512], F32, tag="z", name="zp1")
            # both i-chunks per jt: consecutive matmuls share each stationary
            # operand, halving the LDWEIGHTS issue pressure
            for jt in range(8):
                p0 = ps_s.tile([128, 512], F32, tag="s", name="p0")
                p1 = ps_s.tile([128, 512], F32, tag="s", name="p1")
                for icnk, p in ((0, p0), (1, p1)):
                    nc.tensor.matmul(
                        p[:],
                        ktp[:, h * 1024 + jt * 128: h * 1024 + jt * 128 + 128],
                        quT[h // 2][:, icnk * 512:(icnk + 1) * 512],
                        start=True,
                        stop=False,
                    )
                for cc in range(4):
                    for icnk, p in ((0, p0), (1, p1)):
                        nc.tensor.matmul(
                            p[:],
                            ft[cc][:, jt * 128:(jt + 1) * 128],
                            a_cur[:, cc * 1024 + icnk * 512:
                                  cc * 1024 + icnk * 512 + 512],
                            start=False,
                            stop=(cc == 3),
                        )
                for icnk, p in ((0, p0), (1, p1)):
                    nc.scalar.activation(
                        attnT[:, jt * 1024 + icnk * 512:
                              jt * 1024 + icnk * 512 + 512],
                        p[:], Exp, scale=float(SCALE))

                # spread next-next head's G matmuls: one per jt block (the
                # extra stream time lets the LDWEIGHTS pipeline catch up)
                if h + 2 < H:
                    if jt == 0:
                        g_spread = gpool.tile([128, 4096], DT, tag="g",
                                              name="g_spread")
                    emit_g_piece(h + 2, g_spread, jt)
                    if jt == 7:
                        g_next = g_spread
                if jt == 3 and h + 1 < H:
                    a_next = emit_rope(h + 1, g_cur_next)

            for jt in range(8):
                emit_av_mm(h, 0, jt, zp0, attnT)
            for jt in range(8):
                emit_av_mm(h, 1, jt, zp1, attnT)
            emit_znorm(h, 0, zp0)
            emit_znorm(h, 1, zp1)

            if h + 1 < H:
                a_cur = a_next
            if h + 2 < H:
                g_cur_next = g_next


        # ---- output projection ----
        for it in range(8):
            p = ps_s.tile([128, 512], F32, tag="s")
            for ncnk in range(4):
                nc.tensor.matmul(
                    p[:],
                    zT[ncnk][:, it * 128:(it + 1) * 128],
                    wout[ncnk][:],
                    start=(ncnk == 0),
                    stop=(ncnk == 3),
                )
            osb = opool.tile([128, 512], F32, tag="osb")
            nc.scalar.activation(osb[:], p[:], Copy)
            nc.sync.dma_start(out_d[it * 128:(it + 1) * 128, :], osb[:])

    nc.compile()
    _dedup_ldweights(nc)
    return nc


def _dedup_ldweights(nc):
    """Drop an InstLdweights when the immediately-preceding PE weight load
    (with only matmuls in between) loaded the identical stationary operand.
    Our paired score matmuls reuse each stationary operand twice; the
    duplicate load is what limits the PE instruction issue rate."""
    removed = 0
    for fn in nc.m.functions:
        for blk in fn.blocks:
            last_sig = None
            newlist = []
            for inst in blk.instructions:
                if isinstance(inst, mybir.InstLdweights):
                    sig = str(inst.ins[0])
                    si = inst.sync_info
                    clean = si is None or (
                        len(si.on_wait) == 0 and len(si.on_update) == 0)
                    if clean and sig == last_sig:
                        removed += 1
                        continue
                    last_sig = sig
                    newlist.append(inst)
                else:
                    newlist.append(inst)
            blk.instructions[:] = newlist
    return removed


def make_host_inputs(xs, Wq, Wk, Wv, Wpos, Wout, u_bias, v_bias, mode=MM_MODE):
    """Build the per-core input maps (host-side layout prep only)."""
    npdt = _np_dt(mode)
    kk = np.arange(256, dtype=np.float64)
    omega = np.exp(-np.log(10000.0) * (2.0 * kk) / D)          # (256,)
    ang = np.outer(omega, np.arange(T, dtype=np.float64))      # (256, T)
    sin_t = np.sin(ang).astype(np.float32)
    cos_t = np.cos(ang).astype(np.float32)
    FT = np.concatenate([sin_t, cos_t], axis=0)                # (512, T)

    perm = np.concatenate([np.arange(0, D, 2), np.arange(1, D, 2)])
    WPTn = Wpos[perm, :].T                                     # (hd, c')
    # per-head 128-row zero-padded stationary blocks: head h's 64 rows sit at
    # partition (h%2)*64, rest zero (kills the co-resident head's q rows)
    WPT = np.zeros((2 * D, D), np.float32)
    for h in range(H):
        WPT[h * 128 + (h % 2) * 64: h * 128 + (h % 2) * 64 + 64, :] = \
            WPTn[h * DH:(h + 1) * DH, :]

    ubT = np.ascontiguousarray(
        u_bias.reshape(-1).astype(np.float32).reshape(4, 128).T)
    vbT = np.ascontiguousarray(
        v_bias.reshape(-1).astype(np.float32).reshape(4, 128).T)

    shared = {
        "Wq": np.ascontiguousarray(Wq).astype(npdt),
        "Wk": np.ascontiguousarray(Wk).astype(npdt),
        "Wv": np.ascontiguousarray(Wv).astype(npdt),
        "WPT": WPT.astype(npdt),
        "Wout": np.ascontiguousarray(Wout).astype(npdt),
        "ubT": ubT,
        "vbT": vbT,
        "FT": np.ascontiguousarray(FT).astype(npdt),
        "CT": np.ascontiguousarray(cos_t).astype(npdt),
        "ST": np.ascontiguousarray(sin_t).astype(npdt),
    }
    in_maps = []
    for b in range(B):
        m = dict(shared)
        m["xsT"] = np.ascontiguousarray(xs[b].T).astype(npdt)
        in_maps.append(m)
    return in_maps


_NC_CACHE = {}


def get_nc(mode=MM_MODE):
    if mode not in _NC_CACHE:
        _NC_CACHE[mode] = build_nc(mode)
    return _NC_CACHE[mode]


def _numpy_reference(xs, mask, Wq, Wk, Wv, Wpos, Wout, u_bias, v_bias):
    """Exact (fp32 numpy) fallback for non-all-ones masks."""
    b, t, _ = xs.shape
    pos = np.arange(-(t - 1), t, dtype=np.float32)[:, None]
    inv_freq = np.exp(-np.log(10000.0) *
                      np.arange(0, D, 2, dtype=np.float32) / D)
    angv = pos * inv_freq[None, :]
    pe = np.stack([np.sin(angv), np.cos(angv)], axis=-1).reshape(pos.shape[0], D)
    q = (xs @ Wq).reshape(b, t, H, DH).transpose(0, 2, 1, 3)
    k = (xs @ Wk).reshape(b, t, H, DH).transpose(0, 2, 1, 3)
    v = (xs @ Wv).reshape(b, t, H, DH).transpose(0, 2, 1, 3)
    p = (pe @ Wpos).reshape(-1, H, DH).transpose(1, 0, 2)
    q_u = q + u_bias[None, :, None, :]
    q_v = q + v_bias[None, :, None, :]
    ac = np.einsum("bhtd,bhsd->bhts", q_u, k)
    bd = np.einsum("bhtd,hld->bhtl", q_v, p)
    bdp = np.pad(bd, ((0, 0), (0, 0), (0, 0), (1, 0)))
    l = bd.shape[-1]
    bd = bdp.reshape(b, H, l + 1, t)[:, :, 1:, :].reshape(b, H, t, l)[..., :t]
    scores = (ac + bd) * SCALE
    m = (mask[:, None, :, :] == 0)
    scores = np.where(m, -np.inf, scores)
    scores = scores - scores.max(axis=-1, keepdims=True)
    e = np.exp(scores)
    attn = e / e.sum(axis=-1, keepdims=True)
    attn = np.where(m, 0.0, attn)
    z = np.einsum("bhts,bhsd->bthd", attn, v).reshape(b, t, H * DH)
    return (z @ Wout).astype(np.float32)


def kernel(xs, mask, Wq, Wk, Wv, Wpos, Wout, u_bias, v_bias):
    xs = np.asarray(xs, dtype=np.float32)
    mask = np.asarray(mask)
    Wq = np.asarray(Wq, dtype=np.float32)
    Wk = np.asarray(Wk, dtype=np.float32)
    Wv = np.asarray(Wv, dtype=np.float32)
    Wpos = np.asarray(Wpos, dtype=np.float32)
    Wout = np.asarray(Wout, dtype=np.float32)
    u_bias = np.asarray(u_bias, dtype=np.float32)
    v_bias = np.asarray(v_bias, dtype=np.float32)

    if not np.all(mask != 0):
        # the on-device kernel assumes the (spec-pinned) all-ones mask
        return _numpy_reference(xs, mask, Wq, Wk, Wv, Wpos, Wout, u_bias, v_bias)

    nc = get_nc(MM_MODE)
    in_maps = make_host_inputs(xs, Wq, Wk, Wv, Wpos, Wout, u_bias, v_bias,
                               MM_MODE)
    res = run_bass_kernel_spmd(nc, in_maps, core_ids=list(range(NCORES)))
    out = np.stack([np.asarray(res.results[b]["out"], dtype=np.float32)
                    for b in range(B)], axis=0)
    return out


if __name__ == "__main__":
    # smoke-test: build only
    nc = build_nc()
    print("build ok")



# revision 12
# speedup vs baseline: 1.3820x; 1.3820x over previous
"""Trainium2 Bass kernel for Transformer-XL style relative-position multi-head
self-attention (nn_MultiHeadedSelfAttention_35588099015524).

Sharding: batch (B=8) is data-parallel across the 8 NeuronCores; no collectives.

Math trick: the Transformer-XL relative shift is eliminated exactly via
    sin(w(j-i)) = sin(wj)cos(wi) - cos(wj)sin(wi)
    cos(w(j-i)) = cos(wj)cos(wi) + sin(wj)sin(wi)
so  matrix_bd[i,j] = sum_c A'[i,c] * F[j,c]   (plain matmul, K=512)
where G = q_v @ Wpos_h^T (per head), A' is a RoPE-style per-row rotation of G
(elementwise with constant sin/cos tables), and F is the constant sinusoid
table at positions 0..T-1.  Scores are computed transposed (S^T[j,i]) so that
softmax-normalization sums ride along as an extra ones-column in V, and the
attention output and final projection need no on-device transposes at all.
"""

import sys

sys.path.insert(0, "/opt/trn_rl_repo")

from contextlib import ExitStack  # noqa: E402

import numpy as np  # noqa: E402
import ml_dtypes  # noqa: E402

import concourse.bass as bass  # noqa: E402
from concourse import bacc, library_config  # noqa: E402
import concourse.tile as tile  # noqa: E402
from concourse import mybir  # noqa: E402
from concourse.bass_utils import run_bass_kernel_spmd  # noqa: E402

# Force every ACT function we use (Exp/Ln/Copy) to resolve to the single
# "natural_log_exp_and_others" table set — otherwise the table-load pass
# flip-flops between sets per head (~2.7us per ACT_TABLE_LOAD).
import concourse.hw_specs as _hs  # noqa: E402
import concourse.bacc as _bacc_mod  # noqa: E402

if not getattr(_hs, "_act_tables_pinned", False):
    _orig_gat = _hs.get_activation_tables

    def _pinned_gat(arch):
        tabs = _orig_gat(arch)
        keep = "natural_log_exp_and_others"
        pin = {mybir.ActivationFunctionType.Exp,
               mybir.ActivationFunctionType.Ln,
               mybir.ActivationFunctionType.Copy}
        if keep in tabs and pin <= tabs[keep]:
            for k in tabs:
                if k != keep:
                    tabs[k] = tabs[k] - pin
        return tabs

    _hs.get_activation_tables = _pinned_gat
    _bacc_mod.get_activation_tables = _pinned_gat
    _hs._act_tables_pinned = True

B, T, D = 8, 1024, 512
H, DH = 8, 64
NCORES = 8
SCALE = 1.0 / np.sqrt(DH)

F32 = mybir.dt.float32
BF16 = mybir.dt.bfloat16
F8 = mybir.dt.float8e4
DR = mybir.MatmulPerfMode.DoubleRow

# knob: matmul/elementwise working dtype ("bf16" or "f32r")
MM_MODE = "bf16"


def _np_dt(mode):
    return ml_dtypes.bfloat16 if mode == "bf16" else np.float32


def _mm_dt(mode):
    return BF16 if mode == "bf16" else mybir.dt.float32r


def build_nc(mode=MM_MODE):
    """Build the per-core Bass module (identical program on all 8 cores)."""
    DT = _mm_dt(mode)
    nc = bacc.Bacc("TRN2", target_bir_lowering=False, debug=False)

    # ---- DRAM parameters (per core) ----
    xsT_d = nc.declare_dram_parameter("xsT", [D, T], DT, isOutput=False)
    wq_d = nc.declare_dram_parameter("Wq", [D, D], DT, isOutput=False)
    wk_d = nc.declare_dram_parameter("Wk", [D, D], DT, isOutput=False)
    wv_d = nc.declare_dram_parameter("Wv", [D, D], DT, isOutput=False)
    wpt_d = nc.declare_dram_parameter("WPT", [2 * D, D], DT, isOutput=False)
    wout_d = nc.declare_dram_parameter("Wout", [D, D], DT, isOutput=False)
    ubt_d = nc.declare_dram_parameter("ubT", [128, 4], F32, isOutput=False)
    vbt_d = nc.declare_dram_parameter("vbT", [128, 4], F32, isOutput=False)
    ft_d = nc.declare_dram_parameter("FT", [D, T], F8, isOutput=False)
    ct_d = nc.declare_dram_parameter("CT", [256, T], DT, isOutput=False)
    st_d = nc.declare_dram_parameter("ST", [256, T], DT, isOutput=False)
    out_d = nc.declare_dram_parameter("out", [T, D], F32, isOutput=True)

    Exp = mybir.ActivationFunctionType.Exp
    Copy = mybir.ActivationFunctionType.Copy
    MUL = mybir.AluOpType.mult
    ADD = mybir.AluOpType.add
    SUB = mybir.AluOpType.subtract

    with tile.TileContext(nc) as tc, ExitStack() as ctx:
        cpool = ctx.enter_context(tc.tile_pool(name="consts", bufs=1))
        gpool = ctx.enter_context(tc.tile_pool(name="gwork", bufs=2))
        apool = ctx.enter_context(tc.tile_pool(name="attn", bufs=2))
        opool = ctx.enter_context(tc.tile_pool(name="osb", bufs=4))
        rpool = ctx.enter_context(tc.tile_pool(name="recip", bufs=2))
        ps_s = ctx.enter_context(tc.tile_pool(name="ps_s", bufs=3, space="PSUM"))
        ps_g = ctx.enter_context(tc.tile_pool(name="ps_g", bufs=3, space="PSUM"))
        ps_z = ctx.enter_context(tc.tile_pool(name="ps_z", bufs=2, space="PSUM"))

        # ---- load constants / inputs into SBUF ----
        # one wide tile per tensor, one coalesced DMA (blocks along free dim)
        def load_wide(dram, rows, cols, tag, dt=DT):
            nblk = rows // 128
            t = cpool.tile([128, nblk * cols], dt, tag=tag, name=tag)
            nc.sync.dma_start(
                t[:].rearrange("p (c i) -> p c i", c=nblk),
                dram[:, :].rearrange("(c p) i -> p c i", p=128))
            return [t[:, c * cols:(c + 1) * cols] for c in range(nblk)]

        # PE warm-up during the input-DMA window: 12 dependency-free matmuls
        # all writing ONE psum tile (WAW keeps them in-order on PE; no pool
        # churn), so HAM reaches 8/8 before the first real matmul
        warm = cpool.tile([128, 512], DT, tag="warm", name="warm")
        nc.vector.memset(warm[:], 0.0)
        wp = ps_z.tile([128, 512], F32, tag="z", name="warmp")
        for w in range(12):
            nc.tensor.matmul(wp[:], warm[:, 0:128], warm[:], start=True,
                             stop=True)

        # interleave the first chunks of xsT and Wq so the first projection
        # matmul can issue as early as possible
        xsT_tile = cpool.tile([128, 4 * T], DT, tag="xsT", name="xsT")
        wq_tile = cpool.tile([128, 4 * D], DT, tag="wq", name="wq")
        for c in range(4):
            nc.sync.dma_start(xsT_tile[:, c * T:(c + 1) * T],
                              xsT_d[c * 128:(c + 1) * 128, :])
            nc.sync.dma_start(wq_tile[:, c * D:(c + 1) * D],
                              wq_d[c * 128:(c + 1) * 128, :])
        xsT = [xsT_tile[:, c * T:(c + 1) * T] for c in range(4)]
        wq = [wq_tile[:, c * D:(c + 1) * D] for c in range(4)]
        ubt = cpool.tile([128, 4], F32, tag="ubt")
        nc.sync.dma_start(ubt[:], ubt_d[:, :])
        vbt = cpool.tile([128, 4], F32, tag="vbt")
        nc.sync.dma_start(vbt[:], vbt_d[:, :])
        wpt = load_wide(wpt_d, 2 * D, D, "wpt")
        wk = load_wide(wk_d, D, D, "wk")
        wv = load_wide(wv_d, D, D, "wv")
        ct = load_wide(ct_d, 256, T, "ct")
        st = load_wide(st_d, 256, T, "st")
        ft_t = cpool.tile([128, 4 * T], F8, tag="ft", name="ft")
        nc.sync.dma_start(
            ft_t[:].rearrange("p (c i) -> p c i", c=4),
            ft_d[:, :].rearrange("(c p) i -> p c i", p=128))
        wout = load_wide(wout_d, D, D, "wout")
        # 3D view of the fp8 F table for DoubleRow score matmuls:
        # [128 c-in-tile, 4 c-tiles, 1024 j]
        ft3 = ft_t[:].rearrange("p (c j) -> p c j", c=4)

        # computed persistent tensors
        quT = [cpool.tile([128, T], DT, tag=f"quT{c}", name=f"quT{c}") for c in range(4)]
        qvT = [cpool.tile([128, T], DT, tag=f"qvT{c}", name=f"qvT{c}") for c in range(4)]
        ktp = cpool.tile([128, H * T], DT, tag="ktp", name="ktp")
        zT = [cpool.tile([128, T], DT, tag=f"zT{c}", name=f"zT{c}") for c in range(4)]
        vp = cpool.tile([128, 8 * 520], DT, tag="vp")

        # gpsimd ucode library providing InstPartitionBroadcast
        nc.gpsimd.load_library(library_config.attn)
        # ones columns for the softmax-sum trick (V gets overwritten on top)
        nc.gpsimd.memset(vp[:], 1.0)

        # ---- projections ----
        # Q^T[n,i] = sum_d Wq[d,n] xsT[d,i]   (psum tile per (n-chunk, i-chunk))
        for nchunk in range(4):
            for icnk in range(2):
                p = ps_s.tile([128, 512], F32, tag="s")
                for kc in range(4):
                    nc.tensor.matmul(
                        p[:],
                        wq[kc][:, nchunk * 128:(nchunk + 1) * 128],
                        xsT[kc][:, icnk * 512:(icnk + 1) * 512],
                        start=(kc == 0),
                        stop=(kc == 3),
                    )
                dst = quT[nchunk][:, icnk * 512:(icnk + 1) * 512]
                nc.vector.tensor_scalar_add(dst, p[:], ubt[:, nchunk:nchunk + 1])
                dst = qvT[nchunk][:, icnk * 512:(icnk + 1) * 512]
                nc.vector.tensor_scalar_add(dst, p[:], vbt[:, nchunk:nchunk + 1])

        # K^T per head, zero-padded to 128 contraction rows (so score matmuls
        # never change the stationary row count); head h's 64 real rows sit at
        # partitions (h%2)*64 to match quT/qvT tile row coordinates.
        # Projection runs M=128 (head pair 2c,2c+1 stacked); the psum->ktp
        # copies scatter each head's 64 rows into its zero-padded segment.
        nc.gpsimd.memset(ktp[:], 0.0)
        for c in range(4):
            for jc in range(2):
                p = ps_s.tile([128, 512], F32, tag="s")
                for kc in range(4):
                    nc.tensor.matmul(
                        p[:],
                        wk[kc][:, c * 128:(c + 1) * 128],
                        xsT[kc][:, jc * 512:(jc + 1) * 512],
                        start=(kc == 0),
                        stop=(kc == 3),
                    )
                for hh in range(2):
                    h = 2 * c + hh
                    row = hh * 64
                    dst = ktp[row:row + 64, h * 1024 + jc * 512:
                              h * 1024 + jc * 512 + 512]
                    nc.scalar.activation(dst, p[row:row + 64, :], Copy)

        # V[j,n] = sum_d xsT[d,j] Wv[d,n]; store with stride 65 into vp
        for jt in range(8):
            p = ps_s.tile([128, 512], F32, tag="s")
            for kc in range(4):
                nc.tensor.matmul(
                    p[:],
                    xsT[kc][:, jt * 128:(jt + 1) * 128],
                    wv[kc][:],
                    start=(kc == 0),
                    stop=(kc == 3),
                )
            dst = vp[:, jt * 520:(jt + 1) * 520].rearrange(
                "p (h x) -> p h x", h=8)[:, :, 0:64]
            src = p[:].rearrange("p (h x) -> p h x", h=8)
            nc.scalar.activation(dst, src, Copy)

        # ---- per-head G -> rope(A') pipeline ----
        def emit_g_piece(h, g, piece):
            """One G^T matmul (of 8) for head h; piece = cc*2 + icnk."""
            row = (h % 2) * 64
            cc, icnk = piece // 2, piece % 2
            p = ps_g.tile([128, 512], F32, tag="g", name="gp")
            nc.tensor.matmul(
                p[:],
                wpt[h][:, cc * 128:(cc + 1) * 128],
                qvT[h // 2][:, icnk * 512:(icnk + 1) * 512],
                start=True,
                stop=True,
            )
            nc.scalar.activation(
                g[:, cc * 1024 + icnk * 512: cc * 1024 + icnk * 512 + 512],
                p[:], Copy)

        def emit_g(h):
            g = gpool.tile([128, 4096], DT, tag="g")
            for piece in range(8):
                emit_g_piece(h, g, piece)
            return g

        def emit_rope(h, g):
            # A' is produced in fp8e4 (the DoubleRow score matmuls read it);
            # the two mults stay bf16 (2x DVE), only the final add/sub pays
            # the 1-byte-out penalty.
            ap = gpool.tile([128, 4096], F8, tag="aprime")
            tmp = gpool.tile([128, T], DT, tag="tmp1")
            tmp2 = gpool.tile([128, T], DT, tag="tmp2")
            for cc in range(4):
                freq = cc % 2          # which 128-block of frequencies
                sin_blk = cc < 2       # first half pairs with sin(w j)
                ga = g[:, cc * 1024:(cc + 1) * 1024]
                # partner chunk: cc+2 mod 4 (sin block <-> cos block)
                pc = (cc + 2) % 4
                gb = g[:, pc * 1024:(pc + 1) * 1024]
                dst = ap[:, cc * 1024:(cc + 1) * 1024]
                # sin block: A' = Gs*cos + Gc*sin ; cos block: A' = Gc*cos - Gs*sin
                nc.vector.tensor_tensor(tmp[:], ga, ct[freq][:], op=MUL)
                nc.vector.tensor_tensor(tmp2[:], gb, st[freq][:], op=MUL)
                nc.vector.tensor_tensor(dst, tmp[:], tmp2[:],
                                        op=(ADD if sin_blk else SUB))
            return ap

        g_cur = emit_g(0)
        a_cur = emit_rope(0, g_cur)
        g_cur_next = emit_g(1)

        def emit_av_mm(h, icnk, jt, zp, attnT):
            nc.tensor.matmul(
                zp[:],
                vp[:, jt * 520 + 65 * h: jt * 520 + 65 * h + 65],
                attnT[:, jt * 1024 + icnk * 512:
                      jt * 1024 + icnk * 512 + 512],
                start=(jt == 0),
                stop=(jt == 7),
            )

        Ln = mybir.ActivationFunctionType.Ln

        def emit_znorm(h, icnk, zp):
            # 1/s computed as exp(-ln s) on ACT (DVE reciprocal is 8 cyc/elem)
            row = (h % 2) * 64
            lns = rpool.tile([1, 512], F32, tag="lns")
            nc.scalar.activation(lns[:], zp[64:65, :], Ln)
            rec = rpool.tile([1, 512], F32, tag="rec")
            nc.scalar.activation(rec[:], lns[:], Exp, scale=-1.0)
            recb = rpool.tile([64, 512], F32, tag="recb")
            nc.gpsimd.partition_broadcast(recb[:], rec[0:1, :])
            dst = zT[h // 2][row:row + 64, icnk * 512:(icnk + 1) * 512]
            nc.vector.tensor_tensor(dst, zp[0:64, :], recb[:], op=MUL)

        for h in range(H):
            row = (h % 2) * 64
            attnT = apool.tile([128, 8192], DT, tag="attnT")
            zp0 = ps_z.tile([65, 512], F32, tag="z", name="zp0")
            zp1 = ps_z.tile([65, 512], F32, tag="z", name="zp1")
            # both i-chunks per jt: consecutive matmuls share each stationary
            # operand, halving the LDWEIGHTS issue pressure
            a3 = a_cur[:].rearrange("p (c i) -> p c i", c=4)
            for jt in range(8):
                p0 = ps_s.tile([128, 512], F32, tag="s", name="p0")
                p1 = ps_s.tile([128, 512], F32, tag="s", name="p1")
                for icnk, p in ((0, p0), (1, p1)):
                    nc.tensor.matmul(
                        p[:],
                        ktp[:, h * 1024 + jt * 128: h * 1024 + jt * 128 + 128],
                        quT[h // 2][:, icnk * 512:(icnk + 1) * 512],
                        start=True,
                        stop=False,
                    )
                # BD term: fp8 DoubleRow, 2 c-tiles per instruction -> 2
                # matmuls instead of 4 cover the K=512 sinusoid contraction
                for cp in range(2):
                    for icnk, p in ((0, p0), (1, p1)):
                        nc.tensor.matmul(
                            p[:],
                            ft3[:, 2 * cp:2 * cp + 2, jt * 128:(jt + 1) * 128],
                            a3[:, 2 * cp:2 * cp + 2,
                               icnk * 512:(icnk + 1) * 512],
                            start=False,
                            stop=(cp == 1),
                            perf_mode=DR,
                        )
                for icnk, p in ((0, p0), (1, p1)):
                    nc.scalar.activation(
                        attnT[:, jt * 1024 + icnk * 512:
                              jt * 1024 + icnk * 512 + 512],
                        p[:], Exp, scale=float(SCALE))

                # spread next-next head's G matmuls: one per jt block (the
                # extra stream time lets the LDWEIGHTS pipeline catch up)
                if h + 2 < H:
                    if jt == 0:
                        g_spread = gpool.tile([128, 4096], DT, tag="g",
                                              name="g_spread")
                    emit_g_piece(h + 2, g_spread, jt)
                    if jt == 7:
                        g_next = g_spread
                if jt == 3 and h + 1 < H:
                    a_next = emit_rope(h + 1, g_cur_next)

            # both i-chunks per jt so consecutive AV matmuls share the
            # stationary V block (ldweights dedup)
            for jt in range(8):
                emit_av_mm(h, 0, jt, zp0, attnT)
                emit_av_mm(h, 1, jt, zp1, attnT)
            emit_znorm(h, 0, zp0)
            emit_znorm(h, 1, zp1)

            if h + 1 < H:
                a_cur = a_next
            if h + 2 < H:
                g_cur_next = g_next


        # ---- output projection ----
        for it in range(8):
            p = ps_s.tile([128, 512], F32, tag="s")
            for ncnk in range(4):
                nc.tensor.matmul(
                    p[:],
                    zT[ncnk][:, it * 128:(it + 1) * 128],
                    wout[ncnk][:],
                    start=(ncnk == 0),
                    stop=(ncnk == 3),
                )
            osb = opool.tile([128, 512], F32, tag="osb")
            nc.scalar.activation(osb[:], p[:], Copy)
            nc.sync.dma_start(out_d[it * 128:(it + 1) * 128, :], osb[:])

    nc.compile()
    _dedup_ldweights(nc)
    return nc


def _dedup_ldweights(nc):
    """Drop an InstLdweights when the immediately-preceding PE weight load
    (with only matmuls in between) loaded the identical stationary operand.
    Our paired score matmuls reuse each stationary operand twice; the
    duplicate load is what limits the PE instruction issue rate."""
    removed = 0
    for fn in nc.m.functions:
        for blk in fn.blocks:
            last_sig = None
            newlist = []
            for inst in blk.instructions:
                if isinstance(inst, mybir.InstLdweights):
                    sig = f"{inst.ins[0]}|{inst.perf_mode}"
                    si = inst.sync_info
                    clean = si is None or (
                        len(si.on_wait) == 0 and len(si.on_update) == 0)
                    if clean and sig == last_sig:
                        removed += 1
                        continue
                    last_sig = sig
                    newlist.append(inst)
                else:
                    newlist.append(inst)
            blk.instructions[:] = newlist
    return removed


def make_host_inputs(xs, Wq, Wk, Wv, Wpos, Wout, u_bias, v_bias, mode=MM_MODE):
    """Build the per-core input maps (host-side layout prep only)."""
    npdt = _np_dt(mode)
    kk = np.arange(256, dtype=np.float64)
    omega = np.exp(-np.log(10000.0) * (2.0 * kk) / D)          # (256,)
    ang = np.outer(omega, np.arange(T, dtype=np.float64))      # (256, T)
    sin_t = np.sin(ang).astype(np.float32)
    cos_t = np.cos(ang).astype(np.float32)
    FT = np.concatenate([sin_t, cos_t], axis=0)                # (512, T)

    perm = np.concatenate([np.arange(0, D, 2), np.arange(1, D, 2)])
    WPTn = Wpos[perm, :].T                                     # (hd, c')
    # per-head 128-row zero-padded stationary blocks: head h's 64 rows sit at
    # partition (h%2)*64, rest zero (kills the co-resident head's q rows)
    WPT = np.zeros((2 * D, D), np.float32)
    for h in range(H):
        WPT[h * 128 + (h % 2) * 64: h * 128 + (h % 2) * 64 + 64, :] = \
            WPTn[h * DH:(h + 1) * DH, :]

    ubT = np.ascontiguousarray(
        u_bias.reshape(-1).astype(np.float32).reshape(4, 128).T)
    vbT = np.ascontiguousarray(
        v_bias.reshape(-1).astype(np.float32).reshape(4, 128).T)

    shared = {
        "Wq": np.ascontiguousarray(Wq).astype(npdt),
        "Wk": np.ascontiguousarray(Wk).astype(npdt),
        "Wv": np.ascontiguousarray(Wv).astype(npdt),
        "WPT": WPT.astype(npdt),
        "Wout": np.ascontiguousarray(Wout).astype(npdt),
        "ubT": ubT,
        "vbT": vbT,
        "FT": np.ascontiguousarray(FT).astype(ml_dtypes.float8_e4m3),
        "CT": np.ascontiguousarray(cos_t).astype(npdt),
        "ST": np.ascontiguousarray(sin_t).astype(npdt),
    }
    in_maps = []
    for b in range(B):
        m = dict(shared)
        m["xsT"] = np.ascontiguousarray(xs[b].T).astype(npdt)
        in_maps.append(m)
    return in_maps


_NC_CACHE = {}


def get_nc(mode=MM_MODE):
    if mode not in _NC_CACHE:
        _NC_CACHE[mode] = build_nc(mode)
    return _NC_CACHE[mode]


def _numpy_reference(xs, mask, Wq, Wk, Wv, Wpos, Wout, u_bias, v_bias):
    """Exact (fp32 numpy) fallback for non-all-ones masks."""
    b, t, _ = xs.shape
    pos = np.arange(-(t - 1), t, dtype=np.float32)[:, None]
    inv_freq = np.exp(-np.log(10000.0) *
                      np.arange(0, D, 2, dtype=np.float32) / D)
    angv = pos * inv_freq[None, :]
    pe = np.stack([np.sin(angv), np.cos(angv)], axis=-1).reshape(pos.shape[0], D)
    q = (xs @ Wq).reshape(b, t, H, DH).transpose(0, 2, 1, 3)
    k = (xs @ Wk).reshape(b, t, H, DH).transpose(0, 2, 1, 3)
    v = (xs @ Wv).reshape(b, t, H, DH).transpose(0, 2, 1, 3)
    p = (pe @ Wpos).reshape(-1, H, DH).transpose(1, 0, 2)
    q_u = q + u_bias[None, :, None, :]
    q_v = q + v_bias[None, :, None, :]
    ac = np.einsum("bhtd,bhsd->bhts", q_u, k)
    bd = np.einsum("bhtd,hld->bhtl", q_v, p)
    bdp = np.pad(bd, ((0, 0), (0, 0), (0, 0), (1, 0)))
    l = bd.shape[-1]
    bd = bdp.reshape(b, H, l + 1, t)[:, :, 1:, :].reshape(b, H, t, l)[..., :t]
    scores = (ac + bd) * SCALE
    m = (mask[:, None, :, :] == 0)
    scores = np.where(m, -np.inf, scores)
    scores = scores - scores.max(axis=-1, keepdims=True)
    e = np.exp(scores)
    attn = e / e.sum(axis=-1, keepdims=True)
    attn = np.where(m, 0.0, attn)
    z = np.einsum("bhts,bhsd->bthd", attn, v).reshape(b, t, H * DH)
    return (z @ Wout).astype(np.float32)


def kernel(xs, mask, Wq, Wk, Wv, Wpos, Wout, u_bias, v_bias):
    xs = np.asarray(xs, dtype=np.float32)
    mask = np.asarray(mask)
    Wq = np.asarray(Wq, dtype=np.float32)
    Wk = np.asarray(Wk, dtype=np.float32)
    Wv = np.asarray(Wv, dtype=np.float32)
    Wpos = np.asarray(Wpos, dtype=np.float32)
    Wout = np.asarray(Wout, dtype=np.float32)
    u_bias = np.asarray(u_bias, dtype=np.float32)
    v_bias = np.asarray(v_bias, dtype=np.float32)

    if not np.all(mask != 0):
        # the on-device kernel assumes the (spec-pinned) all-ones mask
        return _numpy_reference(xs, mask, Wq, Wk, Wv, Wpos, Wout, u_bias, v_bias)

    nc = get_nc(MM_MODE)
    in_maps = make_host_inputs(xs, Wq, Wk, Wv, Wpos, Wout, u_bias, v_bias,
                               MM_MODE)
    res = run_bass_kernel_spmd(nc, in_maps, core_ids=list(range(NCORES)))
    out = np.stack([np.asarray(res.results[b]["out"], dtype=np.float32)
                    for b in range(B)], axis=0)
    return out


if __name__ == "__main__":
    # smoke-test: build only
    nc = build_nc()
    print("build ok")



# revision 20
# speedup vs baseline: 1.4659x; 1.0607x over previous
"""Trainium2 Bass kernel for Transformer-XL style relative-position multi-head
self-attention (nn_MultiHeadedSelfAttention_35588099015524).

Sharding: batch (B=8) is data-parallel across the 8 NeuronCores; no collectives.

Math trick: the Transformer-XL relative shift is eliminated exactly via
    sin(w(j-i)) = sin(wj)cos(wi) - cos(wj)sin(wi)
    cos(w(j-i)) = cos(wj)cos(wi) + sin(wj)sin(wi)
so  matrix_bd[i,j] = sum_c A'[i,c] * F[j,c]   (plain matmul, K=512)
where G = q_v @ Wpos_h^T (per head), A' is a RoPE-style per-row rotation of G
(elementwise with constant sin/cos tables), and F is the constant sinusoid
table at positions 0..T-1.  Scores are computed transposed (S^T[j,i]) so that
softmax-normalization sums ride along as an extra ones-column in V, and the
attention output and final projection need no on-device transposes at all.
"""

import sys

sys.path.insert(0, "/opt/trn_rl_repo")

from contextlib import ExitStack  # noqa: E402

import numpy as np  # noqa: E402
import ml_dtypes  # noqa: E402

import concourse.bass as bass  # noqa: E402
from concourse import bacc, library_config  # noqa: E402
import concourse.tile as tile  # noqa: E402
from concourse import mybir  # noqa: E402
from concourse.bass_utils import run_bass_kernel_spmd  # noqa: E402

# Force every ACT function we use (Exp/Ln/Copy) to resolve to the single
# "natural_log_exp_and_others" table set — otherwise the table-load pass
# flip-flops between sets per head (~2.7us per ACT_TABLE_LOAD).
import concourse.hw_specs as _hs  # noqa: E402
import concourse.bacc as _bacc_mod  # noqa: E402

if not getattr(_hs, "_act_tables_pinned", False):
    _orig_gat = _hs.get_activation_tables

    def _pinned_gat(arch):
        tabs = _orig_gat(arch)
        keep = "natural_log_exp_and_others"
        pin = {mybir.ActivationFunctionType.Exp,
               mybir.ActivationFunctionType.Ln,
               mybir.ActivationFunctionType.Copy}
        if keep in tabs and pin <= tabs[keep]:
            for k in tabs:
                if k != keep:
                    tabs[k] = tabs[k] - pin
        return tabs

    _hs.get_activation_tables = _pinned_gat
    _bacc_mod.get_activation_tables = _pinned_gat
    _hs._act_tables_pinned = True

B, T, D = 8, 1024, 512
H, DH = 8, 64
NCORES = 8
SCALE = 1.0 / np.sqrt(DH)

F32 = mybir.dt.float32
BF16 = mybir.dt.bfloat16
F8 = mybir.dt.float8e4
DR = mybir.MatmulPerfMode.DoubleRow

# knob: matmul/elementwise working dtype ("bf16" or "f32r")
MM_MODE = "bf16"


def _np_dt(mode):
    return ml_dtypes.bfloat16 if mode == "bf16" else np.float32


def _mm_dt(mode):
    return BF16 if mode == "bf16" else mybir.dt.float32r


def build_nc(mode=MM_MODE):
    """Build the per-core Bass module (identical program on all 8 cores)."""
    DT = _mm_dt(mode)
    nc = bacc.Bacc("TRN2", target_bir_lowering=False, debug=False)

    # ---- DRAM parameters (per core) ----
    xsT_d = nc.declare_dram_parameter("xsT", [D, T], DT, isOutput=False)
    wq_d = nc.declare_dram_parameter("Wq", [D, D], DT, isOutput=False)
    wk_d = nc.declare_dram_parameter("Wk", [D, D], DT, isOutput=False)
    wv_d = nc.declare_dram_parameter("Wv", [D, D], DT, isOutput=False)
    wpt_d = nc.declare_dram_parameter("WPT", [2 * D, D], DT, isOutput=False)
    wout_d = nc.declare_dram_parameter("Wout", [D, D], DT, isOutput=False)
    ubt_d = nc.declare_dram_parameter("ubT", [128, 4], F32, isOutput=False)
    vbt_d = nc.declare_dram_parameter("vbT", [128, 4], F32, isOutput=False)
    ft_d = nc.declare_dram_parameter("FT", [D, T], F8, isOutput=False)
    ct_d = nc.declare_dram_parameter("CT", [256, T], DT, isOutput=False)
    st_d = nc.declare_dram_parameter("ST", [256, T], DT, isOutput=False)
    out_d = nc.declare_dram_parameter("out", [T, D], F32, isOutput=True)

    Exp = mybir.ActivationFunctionType.Exp
    Copy = mybir.ActivationFunctionType.Copy
    MUL = mybir.AluOpType.mult
    ADD = mybir.AluOpType.add
    SUB = mybir.AluOpType.subtract

    with tile.TileContext(nc) as tc, ExitStack() as ctx:
        cpool = ctx.enter_context(tc.tile_pool(name="consts", bufs=1))
        gpool = ctx.enter_context(tc.tile_pool(name="gwork", bufs=2))
        apool = ctx.enter_context(tc.tile_pool(name="attn", bufs=2))
        opool = ctx.enter_context(tc.tile_pool(name="osb", bufs=4))
        rpool = ctx.enter_context(tc.tile_pool(name="recip", bufs=2))
        # scores: [128,1024] 2-bank tiles (both i-chunks -> one wide exp)
        ps_s = ctx.enter_context(tc.tile_pool(name="ps_s", bufs=2, space="PSUM"))
        ps_g = ctx.enter_context(tc.tile_pool(name="ps_g", bufs=2, space="PSUM"))
        ps_z = ctx.enter_context(tc.tile_pool(name="ps_z", bufs=2, space="PSUM"))

        # ---- load constants / inputs into SBUF ----
        # one wide tile per tensor, one coalesced DMA (blocks along free dim)
        _wide_tiles = {}

        def load_wide(dram, rows, cols, tag, dt=DT):
            nblk = rows // 128
            t = cpool.tile([128, nblk * cols], dt, tag=tag, name=tag)
            _wide_tiles[tag] = t
            nc.sync.dma_start(
                t[:].rearrange("p (c i) -> p c i", c=nblk),
                dram[:, :].rearrange("(c p) i -> p c i", p=128))
            return [t[:, c * cols:(c + 1) * cols] for c in range(nblk)]

        # PE warm-up during the input-DMA window: 12 dependency-free matmuls
        # all writing ONE psum tile (WAW keeps them in-order on PE; no pool
        # churn), so HAM reaches 8/8 before the first real matmul
        warm = cpool.tile([128, 512], DT, tag="warm", name="warm")
        nc.vector.memset(warm[:], 0.0)
        wp = ps_g.tile([128, 512], F32, tag="g", name="warmp")
        for w in range(12):
            nc.tensor.matmul(wp[:], warm[:, 0:128], warm[:], start=True,
                             stop=True)

        # interleave the first chunks of xsT and Wq so the first projection
        # matmul can issue as early as possible
        xsT_tile = cpool.tile([128, 4 * T], DT, tag="xsT", name="xsT")
        wq_tile = cpool.tile([128, 4 * D], DT, tag="wq", name="wq")
        for c in range(4):
            nc.sync.dma_start(xsT_tile[:, c * T:(c + 1) * T],
                              xsT_d[c * 128:(c + 1) * 128, :])
            nc.sync.dma_start(wq_tile[:, c * D:(c + 1) * D],
                              wq_d[c * 128:(c + 1) * 128, :])
        xsT = [xsT_tile[:, c * T:(c + 1) * T] for c in range(4)]
        wq = [wq_tile[:, c * D:(c + 1) * D] for c in range(4)]
        ubt = cpool.tile([128, 4], F32, tag="ubt")
        nc.sync.dma_start(ubt[:], ubt_d[:, :])
        vbt = cpool.tile([128, 4], F32, tag="vbt")
        nc.sync.dma_start(vbt[:], vbt_d[:, :])
        wpt = load_wide(wpt_d, 2 * D, D, "wpt")
        wk = load_wide(wk_d, D, D, "wk")
        wv = load_wide(wv_d, D, D, "wv")
        ct = load_wide(ct_d, 256, T, "ct")
        st = load_wide(st_d, 256, T, "st")
        ft_t = cpool.tile([128, 4 * T], F8, tag="ft", name="ft")
        nc.sync.dma_start(
            ft_t[:].rearrange("p (c i) -> p c i", c=4),
            ft_d[:, :].rearrange("(c p) i -> p c i", p=128))
        wout = load_wide(wout_d, D, D, "wout")
        # 3D view of the fp8 F table for DoubleRow score matmuls:
        # [128 c-in-tile, 4 c-tiles, 1024 j]
        ft3 = ft_t[:].rearrange("p (c j) -> p c j", c=4)

        # computed persistent tensors
        quT = [cpool.tile([128, T], DT, tag=f"quT{c}", name=f"quT{c}") for c in range(4)]
        qvT = [cpool.tile([128, T], DT, tag=f"qvT{c}", name=f"qvT{c}") for c in range(4)]
        ktp = cpool.tile([128, H * T], DT, tag="ktp", name="ktp")
        zT = [cpool.tile([128, T], DT, tag=f"zT{c}", name=f"zT{c}") for c in range(4)]
        vp = cpool.tile([128, 8 * 520], DT, tag="vp")

        # gpsimd ucode library providing InstPartitionBroadcast
        nc.gpsimd.load_library(library_config.attn)
        # ones columns for the softmax-sum trick (V gets overwritten on top)
        nc.gpsimd.memset(vp[:], 1.0)

        # ---- projections ----
        # Q^T[n,i] = sum_d Wq[d,n] xsT[d,i]   (psum tile per (n-chunk, i-chunk))
        for nchunk in range(4):
            for icnk in range(2):
                p = ps_s.tile([128, 512], F32, tag="s")
                for kc in range(4):
                    nc.tensor.matmul(
                        p[:],
                        wq[kc][:, nchunk * 128:(nchunk + 1) * 128],
                        xsT[kc][:, icnk * 512:(icnk + 1) * 512],
                        start=(kc == 0),
                        stop=(kc == 3),
                    )
                dst = quT[nchunk][:, icnk * 512:(icnk + 1) * 512]
                nc.vector.tensor_scalar_add(dst, p[:], ubt[:, nchunk:nchunk + 1])
                dst = qvT[nchunk][:, icnk * 512:(icnk + 1) * 512]
                nc.vector.tensor_scalar_add(dst, p[:], vbt[:, nchunk:nchunk + 1])

        # K^T per head, zero-padded to 128 contraction rows (so score matmuls
        # never change the stationary row count); head h's 64 real rows sit at
        # partitions (h%2)*64 to match quT/qvT tile row coordinates.
        # Projection runs M=128 (head pair 2c,2c+1 stacked); the psum->ktp
        # copies scatter each head's 64 rows into its zero-padded segment.
        nc.gpsimd.memset(ktp[:], 0.0)
        for c in range(4):
            for jc in range(2):
                p = ps_s.tile([128, 512], F32, tag="s")
                for kc in range(4):
                    nc.tensor.matmul(
                        p[:],
                        wk[kc][:, c * 128:(c + 1) * 128],
                        xsT[kc][:, jc * 512:(jc + 1) * 512],
                        start=(kc == 0),
                        stop=(kc == 3),
                    )
                for hh in range(2):
                    h = 2 * c + hh
                    row = hh * 64
                    dst = ktp[row:row + 64, h * 1024 + jc * 512:
                              h * 1024 + jc * 512 + 512]
                    nc.scalar.activation(dst, p[row:row + 64, :], Copy)

        # V[j,n] = sum_d xsT[d,j] Wv[d,n]; store with stride 65 into vp
        for jt in range(8):
            p = ps_s.tile([128, 512], F32, tag="s")
            for kc in range(4):
                nc.tensor.matmul(
                    p[:],
                    xsT[kc][:, jt * 128:(jt + 1) * 128],
                    wv[kc][:],
                    start=(kc == 0),
                    stop=(kc == 3),
                )
            dst = vp[:, jt * 520:(jt + 1) * 520].rearrange(
                "p (h x) -> p h x", h=8)[:, :, 0:64]
            src = p[:].rearrange("p (h x) -> p h x", h=8)
            nc.scalar.activation(dst, src, Copy)

        # ---- per-head G -> rope(A') pipeline ----
        def emit_g_piece(h, g, piece):
            """One G^T matmul (of 8) for head h; piece = cc*2 + icnk."""
            row = (h % 2) * 64
            cc, icnk = piece // 2, piece % 2
            p = ps_g.tile([128, 512], F32, tag="g", name="gp")
            nc.tensor.matmul(
                p[:],
                wpt[h][:, cc * 128:(cc + 1) * 128],
                qvT[h // 2][:, icnk * 512:(icnk + 1) * 512],
                start=True,
                stop=True,
            )
            dst = g[:, cc * 1024 + icnk * 512: cc * 1024 + icnk * 512 + 512]
            # alternate the psum->sbuf eviction between DVE and ACT so
            # neither becomes the bottleneck
            if piece % 2 == 0:
                nc.vector.tensor_copy(dst, p[:])
            else:
                nc.scalar.activation(dst, p[:], Copy)

        def emit_g(h):
            g = gpool.tile([128, 4096], DT, tag="g")
            for piece in range(8):
                emit_g_piece(h, g, piece)
            return g

        def emit_rope(h, g):
            # A' is produced in fp8e4 (the DoubleRow score matmuls read it);
            # the mults stay bf16 (2x DVE), only the final add/sub pays the
            # 1-byte-out penalty.  Chunks cc=0,1 (and 2,3) are processed as
            # one 2048-wide op each: the [ct0|ct1] wide table lines up with
            # the per-chunk frequency blocks.
            ap = gpool.tile([128, 4096], F8, tag="aprime")
            tmp = gpool.tile([128, 2 * T], DT, tag="tmp1")
            tmp2 = gpool.tile([128, 2 * T], DT, tag="tmp2")
            ctw = _wide_tiles["ct"][:, 0:2048]
            stw = _wide_tiles["st"][:, 0:2048]
            # sin blocks (cc=0,1): A' = Gs*cos + Gc*sin
            nc.vector.tensor_tensor(tmp[:], g[:, 0:2048], ctw, op=MUL)
            nc.vector.tensor_tensor(tmp2[:], g[:, 2048:4096], stw, op=MUL)
            nc.vector.tensor_tensor(ap[:, 0:2048], tmp[:], tmp2[:], op=ADD)
            # cos blocks (cc=2,3): A' = Gc*cos - Gs*sin
            nc.vector.tensor_tensor(tmp[:], g[:, 2048:4096], ctw, op=MUL)
            nc.vector.tensor_tensor(tmp2[:], g[:, 0:2048], stw, op=MUL)
            nc.vector.tensor_tensor(ap[:, 2048:4096], tmp[:], tmp2[:], op=SUB)
            return ap

        g_cur = emit_g(0)
        a_cur = emit_rope(0, g_cur)
        g_cur_next = emit_g(1)

        def emit_av_mm(h, icnk, jt, zp, attnT):
            nc.tensor.matmul(
                zp[:],
                vp[:, jt * 520 + 65 * h: jt * 520 + 65 * h + 65],
                attnT[:, jt * 1024 + icnk * 512:
                      jt * 1024 + icnk * 512 + 512],
                start=(jt == 0),
                stop=(jt == 7),
            )

        Ln = mybir.ActivationFunctionType.Ln

        def emit_znorm(h, icnk, zp):
            # 1/s computed as exp(-ln s) on ACT (DVE reciprocal is 8 cyc/elem)
            row = (h % 2) * 64
            lns = rpool.tile([1, 512], F32, tag="lns")
            nc.scalar.activation(lns[:], zp[64:65, :], Ln)
            rec = rpool.tile([1, 512], F32, tag="rec")
            nc.scalar.activation(rec[:], lns[:], Exp, scale=-1.0)
            recb = rpool.tile([64, 512], F32, tag="recb")
            nc.gpsimd.partition_broadcast(recb[:], rec[0:1, :])
            dst = zT[h // 2][row:row + 64, icnk * 512:(icnk + 1) * 512]
            nc.vector.tensor_tensor(dst, zp[0:64, :], recb[:], op=MUL)

        for h in range(H):
            row = (h % 2) * 64
            attnT = apool.tile([128, 8192], DT, tag="attnT")
            zp0 = ps_z.tile([65, 512], F32, tag="z", name="zp0")
            zp1 = ps_z.tile([65, 512], F32, tag="z", name="zp1")
            # both i-chunks per jt: consecutive matmuls share each stationary
            # operand, halving the LDWEIGHTS issue pressure
            a3 = a_cur[:].rearrange("p (c i) -> p c i", c=4)
            for jt in range(8):
                # one 2-bank psum tile holds both i-chunks; each bank is its
                # own accumulation group, evicted by a single wide exp
                sc = ps_s.tile([128, 1024], F32, tag="s", name="sc")
                for icnk in range(2):
                    nc.tensor.matmul(
                        sc[:, icnk * 512:icnk * 512 + 512],
                        ktp[:, h * 1024 + jt * 128: h * 1024 + jt * 128 + 128],
                        quT[h // 2][:, icnk * 512:(icnk + 1) * 512],
                        start=True,
                        stop=False,
                    )
                # BD term: fp8 DoubleRow, 2 c-tiles per instruction -> 2
                # matmuls instead of 4 cover the K=512 sinusoid contraction
                for cp in range(2):
                    for icnk in range(2):
                        nc.tensor.matmul(
                            sc[:, icnk * 512:icnk * 512 + 512],
                            ft3[:, 2 * cp:2 * cp + 2, jt * 128:(jt + 1) * 128],
                            a3[:, 2 * cp:2 * cp + 2,
                               icnk * 512:(icnk + 1) * 512],
                            start=False,
                            stop=(cp == 1),
                            perf_mode=DR,
                        )
                nc.scalar.activation(
                    attnT[:, jt * 1024: jt * 1024 + 1024],
                    sc[:, 0:1024], Exp, scale=float(SCALE))

                # AV matmuls for the previous jt block ride right behind the
                # score groups so the PE never waits on the exp eviction at
                # the end of the head
                if jt >= 1:
                    emit_av_mm(h, 0, jt - 1, zp0, attnT)
                    emit_av_mm(h, 1, jt - 1, zp1, attnT)

                # spread next-next head's G matmuls: one per jt block (the
                # extra stream time lets the LDWEIGHTS pipeline catch up)
                if h + 2 < H:
                    if jt == 0:
                        g_spread = gpool.tile([128, 4096], DT, tag="g",
                                              name="g_spread")
                    emit_g_piece(h + 2, g_spread, jt)
                    if jt == 7:
                        g_next = g_spread
                if jt == 3 and h + 1 < H:
                    a_next = emit_rope(h + 1, g_cur_next)

            emit_av_mm(h, 0, 7, zp0, attnT)
            emit_av_mm(h, 1, 7, zp1, attnT)
            emit_znorm(h, 0, zp0)
            emit_znorm(h, 1, zp1)

            if h + 1 < H:
                a_cur = a_next
            if h + 2 < H:
                g_cur_next = g_next


        # ---- output projection ----
        for it in range(8):
            p = ps_s.tile([128, 512], F32, tag="s")
            for ncnk in range(4):
                nc.tensor.matmul(
                    p[:],
                    zT[ncnk][:, it * 128:(it + 1) * 128],
                    wout[ncnk][:],
                    start=(ncnk == 0),
                    stop=(ncnk == 3),
                )
            osb = opool.tile([128, 512], F32, tag="osb")
            nc.scalar.activation(osb[:], p[:], Copy)
            nc.sync.dma_start(out_d[it * 128:(it + 1) * 128, :], osb[:])

    nc.compile()
    _dedup_ldweights(nc)
    return nc


def _dedup_ldweights(nc):
    """Drop an InstLdweights when the immediately-preceding PE weight load
    (with only matmuls in between) loaded the identical stationary operand.
    Our paired score matmuls reuse each stationary operand twice; the
    duplicate load is what limits the PE instruction issue rate."""
    removed = 0
    for fn in nc.m.functions:
        for blk in fn.blocks:
            last_sig = None
            newlist = []
            for inst in blk.instructions:
                if isinstance(inst, mybir.InstLdweights):
                    sig = f"{inst.ins[0]}|{inst.perf_mode}"
                    si = inst.sync_info
                    clean = si is None or (
                        len(si.on_wait) == 0 and len(si.on_update) == 0)
                    if clean and sig == last_sig:
                        removed += 1
                        continue
                    last_sig = sig
                    newlist.append(inst)
                else:
                    newlist.append(inst)
            blk.instructions[:] = newlist
    return removed


def make_host_inputs(xs, Wq, Wk, Wv, Wpos, Wout, u_bias, v_bias, mode=MM_MODE):
    """Build the per-core input maps (host-side layout prep only)."""
    npdt = _np_dt(mode)
    kk = np.arange(256, dtype=np.float64)
    omega = np.exp(-np.log(10000.0) * (2.0 * kk) / D)          # (256,)
    ang = np.outer(omega, np.arange(T, dtype=np.float64))      # (256, T)
    sin_t = np.sin(ang).astype(np.float32)
    cos_t = np.cos(ang).astype(np.float32)
    FT = np.concatenate([sin_t, cos_t], axis=0)                # (512, T)

    perm = np.concatenate([np.arange(0, D, 2), np.arange(1, D, 2)])
    WPTn = Wpos[perm, :].T                                     # (hd, c')
    # per-head 128-row zero-padded stationary blocks: head h's 64 rows sit at
    # partition (h%2)*64, rest zero (kills the co-resident head's q rows)
    WPT = np.zeros((2 * D, D), np.float32)
    for h in range(H):
        WPT[h * 128 + (h % 2) * 64: h * 128 + (h % 2) * 64 + 64, :] = \
            WPTn[h * DH:(h + 1) * DH, :]

    ubT = np.ascontiguousarray(
        u_bias.reshape(-1).astype(np.float32).reshape(4, 128).T)
    vbT = np.ascontiguousarray(
        v_bias.reshape(-1).astype(np.float32).reshape(4, 128).T)

    shared = {
        "Wq": np.ascontiguousarray(Wq).astype(npdt),
        "Wk": np.ascontiguousarray(Wk).astype(npdt),
        "Wv": np.ascontiguousarray(Wv).astype(npdt),
        "WPT": WPT.astype(npdt),
        "Wout": np.ascontiguousarray(Wout).astype(npdt),
        "ubT": ubT,
        "vbT": vbT,
        "FT": np.ascontiguousarray(FT).astype(ml_dtypes.float8_e4m3),
        "CT": np.ascontiguousarray(cos_t).astype(npdt),
        "ST": np.ascontiguousarray(sin_t).astype(npdt),
    }
    in_maps = []
    for b in range(B):
        m = dict(shared)
        m["xsT"] = np.ascontiguousarray(xs[b].T).astype(npdt)
        in_maps.append(m)
    return in_maps


_NC_CACHE = {}


def get_nc(mode=MM_MODE):
    if mode not in _NC_CACHE:
        _NC_CACHE[mode] = build_nc(mode)
    return _NC_CACHE[mode]


def _numpy_reference(xs, mask, Wq, Wk, Wv, Wpos, Wout, u_bias, v_bias):
    """Exact (fp32 numpy) fallback for non-all-ones masks."""
    b, t, _ = xs.shape
    pos = np.arange(-(t - 1), t, dtype=np.float32)[:, None]
    inv_freq = np.exp(-np.log(10000.0) *
                      np.arange(0, D, 2, dtype=np.float32) / D)
    angv = pos * inv_freq[None, :]
    pe = np.stack([np.sin(angv), np.cos(angv)], axis=-1).reshape(pos.shape[0], D)
    q = (xs @ Wq).reshape(b, t, H, DH).transpose(0, 2, 1, 3)
    k = (xs @ Wk).reshape(b, t, H, DH).transpose(0, 2, 1, 3)
    v = (xs @ Wv).reshape(b, t, H, DH).transpose(0, 2, 1, 3)
    p = (pe @ Wpos).reshape(-1, H, DH).transpose(1, 0, 2)
    q_u = q + u_bias[None, :, None, :]
    q_v = q + v_bias[None, :, None, :]
    ac = np.einsum("bhtd,bhsd->bhts", q_u, k)
    bd = np.einsum("bhtd,hld->bhtl", q_v, p)
    bdp = np.pad(bd, ((0, 0), (0, 0), (0, 0), (1, 0)))
    l = bd.shape[-1]
    bd = bdp.reshape(b, H, l + 1, t)[:, :, 1:, :].reshape(b, H, t, l)[..., :t]
    scores = (ac + bd) * SCALE
    m = (mask[:, None, :, :] == 0)
    scores = np.where(m, -np.inf, scores)
    scores = scores - scores.max(axis=-1, keepdims=True)
    e = np.exp(scores)
    attn = e / e.sum(axis=-1, keepdims=True)
    attn = np.where(m, 0.0, attn)
    z = np.einsum("bhts,bhsd->bthd", attn, v).reshape(b, t, H * DH)
    return (z @ Wout).astype(np.float32)


def kernel(xs, mask, Wq, Wk, Wv, Wpos, Wout, u_bias, v_bias):
    xs = np.asarray(xs, dtype=np.float32)
    mask = np.asarray(mask)
    Wq = np.asarray(Wq, dtype=np.float32)
    Wk = np.asarray(Wk, dtype=np.float32)
    Wv = np.asarray(Wv, dtype=np.float32)
    Wpos = np.asarray(Wpos, dtype=np.float32)
    Wout = np.asarray(Wout, dtype=np.float32)
    u_bias = np.asarray(u_bias, dtype=np.float32)
    v_bias = np.asarray(v_bias, dtype=np.float32)

    if not np.all(mask != 0):
        # the on-device kernel assumes the (spec-pinned) all-ones mask
        return _numpy_reference(xs, mask, Wq, Wk, Wv, Wpos, Wout, u_bias, v_bias)

    nc = get_nc(MM_MODE)
    in_maps = make_host_inputs(xs, Wq, Wk, Wv, Wpos, Wout, u_bias, v_bias,
                               MM_MODE)
    res = run_bass_kernel_spmd(nc, in_maps, core_ids=list(range(NCORES)))
    out = np.stack([np.asarray(res.results[b]["out"], dtype=np.float32)
                    for b in range(B)], axis=0)
    return out


if __name__ == "__main__":
    # smoke-test: build only
    nc = build_nc()
    print("build ok")

